# revision 1
# baseline (speedup 1.0000x reference)
"""Trainium2 Bass kernel for pointer-generator additive attention.

Full op (per batch b):
    dec_fea = s_t_hat @ W_d.T + b_d                         # (n,)
    att     = EF[b] + dec_fea[None,:] + cov[b][:,None]*W_c  # (t, n)
    score   = tanh(att) @ v                                 # (t,)
    attn    = renorm(softmax(score) * mask)                 # (t,)
    c_t     = attn @ EO[b]                                  # (n,)
    cov_next= cov + attn

Data-parallel over batch across 8 NeuronCores (8 batches/core, params
replicated, no collectives).  Measured ~172 us vs ~220-257 us for the
identity-matmul baseline.  Key design points:

  - T is tiled in chunks of 119 rows so K = 119 (identity band)
    + 8 (one-hot dec-row selectors) + 1 (cov -> W_c) = 128: a SINGLE
    fused matmul per (chunk, n-half) computes EF + dec_fea + cov*W_c.
    The lhsT matrix (identity + one-hots + cov row) is built on the
    host; dec rows live at partitions 119..126 of the EF buffers
    (restored by one small DMA per batch since the EF block DMA
    overwrites them), W_c at partition 127 (staged by the host).
  - EF and the att-matmul lhsT are staged in FP8-E4M3 (EO in bf16):
    HBM traffic drops to ~30 MB/core and measured rel_err is 1.3e-2
    against the 2e-2 tolerance.  EO stays bf16 (fp8 EO would put c_t
    at ~4.5e-2).
  - DMA pattern rule learned by microbenchmark: transfers must write
    ALL 128 partitions with p-minor (row-interleaved) DRAM order --
    that runs at ~340-370 B/ns; 119-partition or per-partition-
    contiguous patterns run at ~100-160 B/ns.  EF/EO are therefore
    staged as [9, 128, N] zero-padded blocks.
  - ScalarE: one tanh per chunk (PSUM f32 -> SBUF bf16); VectorE:
    scalar_tensor_tensor with accum_out gives the n-reduction per
    chunk.  Scores regroup to [1, T] rows via one PE transpose and
    two small DMAs per batch.
  - Masked softmax + renorm + coverage batched in GROUPS OF 4 on
    [4, 1024] row tiles (engine partition bases must be 32-aligned,
    so per-group tiles, not [8, T] slices).
  - c_t: per chunk, lhsT = [119, 8] one-hot-column attn (column b =
    attn chunk, rest zeros) so all 8 batches accumulate into a single
    [8, 512] x2 PSUM group; one ScalarE copy + one DMA at the end.
  - Schedule: pass-1 group 0 -> softmax(0) -> {pass-1 group 1
    interleaved with pass-2 group 0} -> softmax(1) -> pass-2 group 1,
    with EF prefetched 2 batches and EO 2-3 buffers ahead, so the PE
    and the DMA engines stay busy across the softmax barriers.
"""

import sys

if "/opt/trn_rl_repo" not in sys.path:
    sys.path.insert(0, "/opt/trn_rl_repo")

import ml_dtypes
import numpy as np

import concourse.bass as bass
import concourse.mybir as mybir
import concourse.tile as tile
from concourse import bacc
from concourse.bass_utils import run_bass_kernel_spmd
from concourse.masks import make_identity

F32 = mybir.dt.float32
BF16 = mybir.dt.bfloat16
FP8 = mybir.dt.float8e4
AF = mybir.ActivationFunctionType
ALU = mybir.AluOpType

N_CORES = 8
B = 64
NB = B // N_CORES  # local batches per core
T = 1024
N = 1024
CH = 119           # t-chunk height (identity rows in the fused matmul)
NCH = 9            # chunks per batch: 8*119 + 72
LAST = T - (NCH - 1) * CH  # 72
W = NCH * CH       # 1071: per-batch window stride in lhsT
KT = N // 128      # k-tiles for the W_d matvec
GRP = 4            # softmax group size


def build_bass(nb: int = NB) -> bass.Bass:
    nc = bacc.Bacc()

    ef_d = nc.declare_dram_parameter("ef_blk", [nb, NCH, 128, N], FP8, isOutput=False)
    eo_d = nc.declare_dram_parameter("eo_blk", [nb, 8, 128, N], BF16, isOutput=False)
    lhsA_d = nc.declare_dram_parameter("lhsT_a", [4, 128, 2048], FP8, isOutput=False)
    lhsB_d = nc.declare_dram_parameter("lhsT_b", [128, nb * W - 8192], FP8, isOutput=False)
    mk_d = nc.declare_dram_parameter("enc_padding_mask", [nb, T], F32, isOutput=False)
    cv_d = nc.declare_dram_parameter("coverage", [nb, T], F32, isOutput=False)
    wdt_d = nc.declare_dram_parameter("W_d_T", [4, 128, 2048], BF16, isOutput=False)
    st_d = nc.declare_dram_parameter("s_t_hat_T", [N, nb], BF16, isOutput=False)
    bd_d = nc.declare_dram_parameter("b_d", [N], BF16, isOutput=False)
    wc_d = nc.declare_dram_parameter("W_c", [N], BF16, isOutput=False)
    v_d = nc.declare_dram_parameter("v", [N], BF16, isOutput=False)
    ct_o = nc.declare_dram_parameter("c_t", [nb, N], F32, isOutput=True)
    at_o = nc.declare_dram_parameter("attn", [nb, T], F32, isOutput=True)
    cn_o = nc.declare_dram_parameter("coverage_next", [nb, T], F32, isOutput=True)

    with tile.TileContext(nc) as tc:
        with (
            tc.tile_pool(name="consts", bufs=1) as consts,
            tc.tile_pool(name="lhsp", bufs=1) as lhsp,
            tc.tile_pool(name="wdtp", bufs=4) as wdtp,
            tc.tile_pool(name="efp", bufs=3) as efp,
            tc.tile_pool(name="eop", bufs=3) as eop,
            tc.tile_pool(name="thp", bufs=4) as thp,
            tc.tile_pool(name="ttro", bufs=3) as ttro,
            tc.tile_pool(name="smal", bufs=2) as smal,
            tc.tile_pool(name="a9p", bufs=2) as a9p,
            tc.tile_pool(name="acwp", bufs=8) as acwp,
            tc.tile_pool(name="psA", bufs=2, space="PSUM") as psA,
            tc.tile_pool(name="psS", bufs=2, space="PSUM") as psS,
            tc.tile_pool(name="psT", bufs=2, space="PSUM") as psT,
        ):
            # ---------------- constants / small inputs ----------------
            ident = consts.tile([CH, CH], F32)
            make_identity(nc, ident)

            # dec matvec inputs stream first on the sync queue so the
            # dec -> restore(0) chain clears as early as possible
            sT_all = consts.tile([128, KT, NB], BF16)
            nc.sync.dma_start(
                out=sT_all, in_=st_d.rearrange("(kj p) b -> p kj b", p=128)
            )
            wpairs = []
            for c2 in range(4):
                wpair = wdtp.tile([128, 2, N], BF16, tag="wp", name=f"wp{c2}")
                nc.sync.dma_start(
                    out=wpair, in_=wdt_d[c2, :, :].rearrange("p (k n) -> p k n", k=2)
                )
                wpairs.append(wpair)

            bd_b = consts.tile([1, N], BF16)
            nc.sync.dma_start(out=bd_b, in_=bd_d[None, :])
            v_b = consts.tile([1, N], BF16)
            nc.sync.dma_start(out=v_b, in_=v_d[None, :])
            wc_b = consts.tile([1, N], BF16)
            nc.sync.dma_start(out=wc_b, in_=wc_d[None, :])
            ones8 = consts.tile([1, NB], BF16)
            nc.vector.memset(ones8, 1.0)

            # first-batch EF blocks load before anything else on gpsimd,
            # then batch 1 in full, then the lhsT constant
            ef_bufs_early = [
                efp.tile([128, NCH, N], FP8, tag="ef", name=f"efb{i}e")
                for i in range(2)]
            for c in range(NCH):
                nc.gpsimd.dma_start(out=ef_bufs_early[0][:, c, :],
                                    in_=ef_d[0, c, :, :])

            # full lhsT (identity band + one-hot dec selectors + cov row)
            lhs_all = lhsp.tile([128, nb * W], FP8)
            nc.gpsimd.dma_start(
                out=lhs_all[:, 0:8192].rearrange("p (c m) -> p c m", c=4),
                in_=lhsA_d[:, :, :].rearrange("c p m -> p c m"),
            )
            nc.gpsimd.dma_start(out=lhs_all[:, 8192:], in_=lhsB_d[:, :])

            nc.gpsimd.dma_start(
                out=ef_bufs_early[1][:, :, :],
                in_=ef_d[1, :, :, :].rearrange("c p n -> p c n"),
            )

            # softmax row tiles: set A serves batches 0-3 and is then
            # reused for batches 6-7 (their masks/coverage load separately);
            # set B serves batches 4-5.  Engine partition bases must be
            # 32-aligned, so every set starts at partition 0.
            mask_A = consts.tile([GRP, T], F32)
            nc.sync.dma_start(out=mask_A, in_=mk_d[0:4, :])
            cov_A = consts.tile([GRP, T], F32)
            nc.sync.dma_start(out=cov_A, in_=cv_d[0:4, :])
            score_A = consts.tile([GRP, T], F32)
            attn_A = consts.tile([GRP, T], F32)
            covn_A = consts.tile([GRP, T], F32)
            mask_B = consts.tile([2, T], F32)
            nc.sync.dma_start(out=mask_B, in_=mk_d[4:6, :])
            cov_B = consts.tile([2, T], F32)
            nc.sync.dma_start(out=cov_B, in_=cv_d[4:6, :])
            score_B = consts.tile([2, T], F32)
            attn_B = consts.tile([2, T], F32)
            covn_B = consts.tile([2, T], F32)
            mask_C = consts.tile([2, T], F32)
            nc.sync.dma_start(out=mask_C, in_=mk_d[6:8, :])
            cov_C = consts.tile([2, T], F32)
            nc.sync.dma_start(out=cov_C, in_=cv_d[6:8, :])

            def sm_row(b):
                # (tile-row holding batch b's score/attn, row index)
                if b < 4:
                    return score_A, attn_A, b
                if b < 6:
                    return score_B, attn_B, b - 4
                return score_A, attn_A, b - 6

            ct_sb = consts.tile([nb, N], F32)

            # v broadcast to all partitions for the score reduction
            v_bcast = consts.tile([128, N], BF16)
            nc.gpsimd.partition_broadcast(v_bcast, v_b)

            # dec_fea rows = s_t_hat @ W_d.T + b_d  (k-pairs as staged)
            dec_rows = consts.tile([NB, N], BF16)
            psd = [psA.tile([NB, 512], F32, tag="att", name=f"psd{h}")
                   for h in range(2)]
            for c2 in range(4):
                for k in range(2):
                    kj = 2 * c2 + k
                    for h in range(2):
                        nc.tensor.matmul(
                            psd[h],
                            lhsT=sT_all[:, kj, :],
                            rhs=wpairs[c2][:, k, h * 512:(h + 1) * 512],
                            start=(kj == 0), stop=False,
                            skip_group_check=True,
                        )
            for h in range(2):
                sl = slice(h * 512, (h + 1) * 512)
                nc.tensor.matmul(
                    psd[h], lhsT=ones8, rhs=bd_b[0:1, sl],
                    start=False, stop=True, skip_group_check=True,
                )
                nc.scalar.activation(dec_rows[:, sl], psd[h], AF.Copy)

            # EF stream buffers: partitions 119..126 = dec rows,
            # partition 127 = W_c (constant across batches/chunks)
            ef_bufs = ef_bufs_early + [
                efp.tile([128, NCH, N], FP8, tag="ef", name="efb2")]
            eo_bufs_pool = [eop.tile([128, 8, N], BF16, tag="eo", name=f"eob{i}")
                            for i in range(3)]
            # one-hot attn-column tiles: one per batch, zeroed ONCE (a
            # per-batch gpsimd memset costs a ~3 us Q7 drain on the
            # critical c_t path); each tile is written in column b only
            acw_tiles = [acwp.tile([128, 8, NB], BF16, tag="acw",
                                   name=f"acw{i}") for i in range(NB)]
            for t_ in acw_tiles:
                nc.vector.memset(t_, 0.0)
            # dec rows replicated across chunks for the per-batch restore DMA
            dec_wide = consts.tile([NB, NCH, N], FP8)
            for c in range(NCH):
                nc.vector.tensor_copy(dec_wide[:, c, :], dec_rows)

            # ---------------- pass 1: scores ----------------
            def chunk_m(c):
                return LAST if c == NCH - 1 else CH

            score_cols_t = {}

            def phase_a(b):
                buf = ef_bufs[b % 3]
                if b > 1:
                    nc.gpsimd.dma_start(
                        out=buf[:, :, :],
                        in_=ef_d[b, :, :, :].rearrange("c p n -> p c n"),
                    )
                # the block DMA zeroes partitions 119..126; restore dec rows
                # (W_c at partition 127 is staged by the host)
                if b == 0:
                    for c in range(NCH):
                        nc.sync.dma_start(
                            out=buf[119:127, c, :], in_=dec_wide[:, c, :]
                        )
                else:
                    nc.sync.dma_start(out=buf[119:127, :, :], in_=dec_wide)

                score_cols = smal.tile([CH, NCH], F32, tag="scol")
                score_cols_t[b] = score_cols
                for c in range(NCH):
                    m = chunk_m(c)
                    att = psA.tile([CH, N], F32, tag="att")
                    for h in range(2):
                        nc.tensor.matmul(
                            att[0:m, h * 512:(h + 1) * 512],
                            lhsT=lhs_all[:, b * W + c * CH: b * W + c * CH + m],
                            rhs=buf[:, c, h * 512:(h + 1) * 512],
                            start=True, stop=True, skip_group_check=True,
                        )
                    th = thp.tile([CH, N], BF16, tag="th")
                    nc.scalar.activation(th[0:m, :], att[0:m, :], AF.Tanh)
                    scr = ttro.tile([CH, N], BF16, tag="ttro")
                    nc.vector.scalar_tensor_tensor(
                        out=scr[0:m, :], in0=th[0:m, :], scalar=1.0,
                        in1=v_bcast[0:m, :],
                        op0=ALU.mult, op1=ALU.mult,
                        accum_out=score_cols[0:m, c:c + 1],
                    )

                # score columns -> row b of score_all (t = c*119 + p)
                ps9 = psT.tile([NCH, CH], F32, tag="tscratch")
                nc.tensor.matmul(
                    ps9, lhsT=score_cols, rhs=ident, is_transpose=True,
                    start=True, stop=True,
                )
                score9 = smal.tile([NCH, CH], F32, tag="s9")
                nc.scalar.activation(score9, ps9, AF.Copy)
                srow, _, r = sm_row(b)
                nc.gpsimd.dma_start(
                    out=srow[r:r + 1, 0:(NCH - 1) * CH].rearrange(
                        "p (c w) -> p c w", c=NCH - 1),
                    in_=score9[0:NCH - 1, :],
                )
                nc.gpsimd.dma_start(
                    out=srow[r:r + 1, (NCH - 1) * CH:T],
                    in_=score9[NCH - 1:NCH, 0:LAST],
                )

            # masked softmax + renorm + coverage for GRP batches at once
            def softmax(score_t, attn_t, mask_t, cov_t, covn_t, nr):
                # scores are O(1) (|s| < ~3): plain exp is safe, skip max-sub
                sl = slice(0, nr)
                nc.scalar.activation(attn_t[sl, :], score_t[sl, :], AF.Exp)
                ssum = smal.tile([GRP, 1], F32, tag="ssum")
                nc.vector.scalar_tensor_tensor(
                    out=attn_t[sl, :], in0=attn_t[sl, :], scalar=1.0,
                    in1=mask_t[sl, :],
                    op0=ALU.mult, op1=ALU.mult, accum_out=ssum[sl, :],
                )
                rs = smal.tile([GRP, 1], F32, tag="rs")
                nc.vector.reciprocal(rs[sl, :], ssum[sl, :])
                nc.vector.tensor_scalar_mul(attn_t[sl, :], attn_t[sl, :],
                                            rs[sl, :])
                nc.vector.tensor_add(covn_t[sl, :], cov_t[sl, :],
                                     attn_t[sl, :])

            # ---------------- pass 2: context vectors ----------------
            eo_bufs = {}

            def load_eo(b):
                buf = eo_bufs_pool[b % 3]
                nc.sync.dma_start(
                    out=buf[:, :, :],
                    in_=eo_d[b, :, :, :].rearrange("c p n -> p c n"),
                )
                eo_bufs[b] = buf

            ctps = [psS.tile([NB, 512], F32, tag="srow", name=f"ctp{h}")
                    for h in range(2)]

            def phase_c(b):
                # attn row -> [8, 128] -> transpose -> one-hot column b
                # (c_t is free of the 119-chunking: EO uses natural blocks)
                attn8 = a9p.tile([8, 128], F32, tag="attn9")
                _, arow, r = sm_row(b)
                nc.sync.dma_start(
                    out=attn8,
                    in_=arow[r:r + 1, :].rearrange("p (j t) -> p j t", j=8),
                )
                acp = psT.tile([128, 8], F32, tag="tscratch")
                nc.tensor.matmul(
                    acp, lhsT=attn8, rhs=ident[0:8, 0:8],
                    is_transpose=True, start=True, stop=True,
                )
                acw = acw_tiles[b]
                nc.scalar.activation(acw[:, :, b], acp, AF.Copy)

                buf = eo_bufs.pop(b)
                for c in range(8):
                    for h in range(2):
                        nc.tensor.matmul(
                            ctps[h],
                            lhsT=acw[:, c, :],
                            rhs=buf[:, c, h * 512:(h + 1) * 512],
                            start=(b == 0 and c == 0),
                            stop=(b == nb - 1 and c == 7),
                            skip_group_check=True,
                        )

            # ---------------- schedule ----------------
            # softmax split 0-3 / 4-5 / 6-7: C(4),C(5) depend only on the
            # early 4-5 softmax, so the post-A(7) barrier is just batches
            # 6-7's row ops, fully hidden under C(4)/C(5)
            for b in range(4):
                phase_a(b)
                if b >= 2:
                    load_eo(b - 2)
            softmax(score_A, attn_A, mask_A, cov_A, covn_A, 4)
            # rows 0-3 flush now so set A can be reused for batches 6-7
            nc.sync.dma_start(out=at_o[0:4, :], in_=attn_A)
            nc.sync.dma_start(out=cn_o[0:4, :], in_=covn_A)
            phase_a(4)
            phase_c(0)
            load_eo(2)
            phase_a(5)
            phase_c(1)
            load_eo(3)
            softmax(score_B, attn_B, mask_B, cov_B, covn_B, 2)
            phase_a(6)
            phase_c(2)
            load_eo(4)
            phase_a(7)
            phase_c(3)
            load_eo(5)
            load_eo(6)
            softmax(score_A, attn_A, mask_C, cov_C, covn_A, 2)
            phase_c(4)
            load_eo(7)
            for b in range(5, nb):
                phase_c(b)
            nc.sync.dma_start(out=at_o[4:6, :], in_=attn_B)
            nc.sync.dma_start(out=cn_o[4:6, :], in_=covn_B)
            nc.sync.dma_start(out=at_o[6:8, :], in_=attn_A[0:2, :])
            nc.sync.dma_start(out=cn_o[6:8, :], in_=covn_A[0:2, :])

            for h in range(2):
                nc.scalar.activation(
                    ct_sb[:, h * 512:(h + 1) * 512], ctps[h], AF.Copy
                )
            nc.sync.dma_start(out=ct_o[:, :], in_=ct_sb)

    nc.finalize()
    return nc


_CACHE: dict = {}


def _get_nc() -> bass.Bass:
    if "nc" not in _CACHE:
        _CACHE["nc"] = build_bass(NB)
    return _CACHE["nc"]


def _stage_tiles(x, row127=None, dtype=ml_dtypes.bfloat16):
    """[T, N] f32 -> [9, 128, N] blocks: block c rows 0..118 hold
    t = c*119 + p (zero-padded past T); rows 119..126 zero; row 127 =
    `row127` if given (W_c for the EF operand).  DMA-friendly: every
    transfer writes all 128 partitions with p-minor lines."""
    blk = np.zeros((NCH, 128, N), np.float32)
    pad = np.zeros((CH * NCH, N), np.float32)
    pad[:T] = x
    blk[:, 0:CH, :] = pad.reshape(NCH, CH, N)
    if row127 is not None:
        blk[:, 127, :] = row127[None, :]
    return blk.astype(dtype)


def _build_lhs(cov_core):
    """identity band + one-hot rows + cov row, split into 4KB-line part A
    ([4, 128, 2048], cols 0..8191 pair-packed) and tail part B."""
    lhs = np.zeros((128, NB * W), np.float32)
    eye = np.eye(CH, dtype=np.float32)
    lhs[:CH] = np.tile(eye, (1, NB * NCH))
    for j in range(NB):
        lhs[CH + j, j * W:(j + 1) * W] = 1.0
        lhs[127, j * W:j * W + T] = cov_core[j]
    lhs = lhs.astype(ml_dtypes.float8_e4m3)
    a = np.ascontiguousarray(lhs[:, 0:8192].reshape(128, 4, 2048).transpose(1, 0, 2))
    b = np.ascontiguousarray(lhs[:, 8192:])
    return a, b


def make_in_maps(inputs: dict) -> list:
    f = lambda x: np.ascontiguousarray(np.asarray(x), dtype=np.float32)
    s = f(inputs["s_t_hat"])
    eo = f(inputs["encoder_outputs"])
    ef = f(inputs["encoder_feature"]).reshape(B, T, N)
    mk = f(inputs["enc_padding_mask"])
    cv = f(inputs["coverage"])
    wdt = np.ascontiguousarray(f(inputs["W_d"]).T).astype(ml_dtypes.bfloat16)
    # [1024, 1024] -> k-tile pair layout [4, 128, 2048] (4KB DMA lines):
    # wdt_all[p, kj, :] = W_d_T[kj*128 + p, :]
    wdt_pairs = np.ascontiguousarray(
        wdt.reshape(KT, 128, N).transpose(1, 0, 2).reshape(128, 4, 2048)
        .transpose(1, 0, 2)
    )
    bd = f(inputs["b_d"])
    wc = f(inputs["W_c"])
    vv = f(inputs["v"])
    in_maps = []
    for i in range(N_CORES):
        sl = slice(i * NB, (i + 1) * NB)
        ef_blk = np.stack([_stage_tiles(ef[i * NB + j], row127=wc,
                                        dtype=ml_dtypes.float8_e4m3)
                           for j in range(NB)])
        eo_blk = np.ascontiguousarray(
            eo[sl].reshape(NB, 8, 128, N)).astype(ml_dtypes.bfloat16)
        lhs_a, lhs_b = _build_lhs(cv[sl])
        in_maps.append({
            "ef_blk": ef_blk,
            "eo_blk": eo_blk,
            "lhsT_a": lhs_a,
            "lhsT_b": lhs_b,
            "s_t_hat_T": np.ascontiguousarray(s[sl].T).astype(ml_dtypes.bfloat16),
            "enc_padding_mask": mk[sl],
            "coverage": cv[sl],
            "W_d_T": wdt_pairs,
            "b_d": bd.astype(ml_dtypes.bfloat16),
            "W_c": wc.astype(ml_dtypes.bfloat16),
            "v": vv.astype(ml_dtypes.bfloat16),
        })
    return in_maps


def gather_outputs(results: list):
    c_t = np.concatenate([results[i]["c_t"] for i in range(N_CORES)], axis=0)
    attn = np.concatenate([results[i]["attn"] for i in range(N_CORES)], axis=0)
    covn = np.concatenate(
        [results[i]["coverage_next"] for i in range(N_CORES)], axis=0
    )
    return c_t, attn, covn


def kernel(**inputs):
    nc = _get_nc()
    in_maps = make_in_maps(inputs)
    res = run_bass_kernel_spmd(nc, in_maps, core_ids=list(range(N_CORES)))
    return gather_outputs(res.results)



# revision 2
# speedup vs baseline: 1.0150x; 1.0150x over previous
"""Trainium2 Bass kernel v2 for pointer-generator additive attention.

Transposed pass-1 design (vs v1's t-partition layout):

  - EF is staged TRANSPOSED per batch: [8 n-chunks, 128 n-rows, 1024 t]
    fp8 (+ a 9th aux chunk: partition 0 = cov fp8, rest zeros).
  - att^T[n,t] = EF^T + W_c[n]*cov[t] (+ dec[n] later) is ONE fp8
    DoubleRow matmul per (chunk, t-half): k-tile 0 = identity(128) over
    EF^T, k-tile 1 = aux (row0 = W_c slice | row0 = cov).  0.5 cyc/col
    and NO padding rows (v1 burned 12.5% on 119-row chunks).
  - dec[n] enters as the per-partition BIAS of the tanh activation
    (bf16, better than v1's fp8 dec rows), decT built once on-device
    via 8 tiny PE transposes of the dec matvec result.
  - score[t] = sum_n v[n]*tanh^T[n,t] is computed on the PE (lhsT =
    v-chunk [128,1]) accumulating straight into a [1,1024] PSUM row --
    this removes v1's 108us DVE scalar_tensor_tensor bottleneck AND the
    score-column transpose machinery.  Score rows drain via DVE copy +
    tiny SBUF->SBUF DMA into [2,1024] softmax group tiles.
  - PSUM: 2x att [128,1024] (4 banks) + shared [98,1024] (2 banks:
    c_t accum rows 0-7, score rows at partition 32 / 64 by batch
    parity) + transpose scratch.
  - c_t: v1 scheme (one-hot attn columns, EO natural layout, all 8
    batches accumulate into shared[0:8,:]).
  - Software-pipelined schedule: step k runs att(k) | score(k,c-2) |
    c_t(k-3) slot-interleaved on the PE, tanh-paced (~9.2us/batch);
    masked softmax in groups of 2; EF/EO prefetched 2-3 steps ahead on
    three DMA queues (sync: EF + odd EO; vector: even EO + W_d;
    gpsimd: smalls).
"""

import sys

if "/opt/trn_rl_repo" not in sys.path:
    sys.path.insert(0, "/opt/trn_rl_repo")

import ml_dtypes
import numpy as np

import concourse.bass as bass
import concourse.mybir as mybir
import concourse.tile as tile
from concourse import bacc
from concourse.bass_utils import run_bass_kernel_spmd
from concourse.masks import make_identity

F32 = mybir.dt.float32
BF16 = mybir.dt.bfloat16
FP8 = mybir.dt.float8e4
AF = mybir.ActivationFunctionType
ALU = mybir.AluOpType
DR = mybir.MatmulPerfMode.DoubleRow

N_CORES = 8
B = 64
NB = B // N_CORES  # 8 local batches per core
T = 1024
N = 1024
NCH = 8            # n-chunks of 128 per batch


def build_bass(nb: int = NB) -> bass.Bass:
    nc = bacc.Bacc()

    ef_d = nc.declare_dram_parameter("ef_blk", [nb, NCH, 128, T], FP8, isOutput=False)
    cv8_d = nc.declare_dram_parameter("cov_fp8", [nb, T], FP8, isOutput=False)
    lhs_d = nc.declare_dram_parameter("lhs_blk", [128, 9, 128], FP8, isOutput=False)
    eo_d = nc.declare_dram_parameter("eo_blk", [nb, 8, 128, N], BF16, isOutput=False)
    mk_d = nc.declare_dram_parameter("enc_padding_mask", [nb, T], F32, isOutput=False)
    cv_d = nc.declare_dram_parameter("coverage", [nb, T], F32, isOutput=False)
    wdt_d = nc.declare_dram_parameter("W_d_T", [4, 128, 2048], BF16, isOutput=False)
    st_d = nc.declare_dram_parameter("s_t_hat_T", [N, nb], BF16, isOutput=False)
    bd_d = nc.declare_dram_parameter("b_d", [N], BF16, isOutput=False)
    vt_d = nc.declare_dram_parameter("v_T", [128, 8], BF16, isOutput=False)
    ct_o = nc.declare_dram_parameter("c_t", [nb, N], F32, isOutput=True)
    at_o = nc.declare_dram_parameter("attn", [nb, T], F32, isOutput=True)
    cn_o = nc.declare_dram_parameter("coverage_next", [nb, T], F32, isOutput=True)

    with tile.TileContext(nc) as tc:
        with (
            tc.tile_pool(name="consts", bufs=1) as consts,
            tc.tile_pool(name="efp", bufs=1) as efp,
            tc.tile_pool(name="eop", bufs=1) as eop,
            tc.tile_pool(name="thp", bufs=1) as thp,
            tc.tile_pool(name="psA", bufs=1, space="PSUM") as psA,
            tc.tile_pool(name="psS", bufs=1, space="PSUM") as psS,
            tc.tile_pool(name="psT", bufs=1, space="PSUM") as psT,
        ):
            # ---------------- SBUF tiles ----------------
            ef_bufs = [efp.tile([128, 9, T], FP8, name=f"ef{i}") for i in range(3)]
            eo_bufs = [eop.tile([128, 8, N], BF16, name=f"eo{i}") for i in range(4)]
            th_bufs = [thp.tile([128, N], BF16, name=f"th{i}") for i in range(4)]

            lhs_t = consts.tile([128, 9, 128], FP8)
            sT_all = consts.tile([128, NCH, NB], BF16)
            wpairs = [consts.tile([128, 2, N], BF16, name=f"wp{i}") for i in range(4)]
            bd_b = consts.tile([1, N], BF16)
            vt_sb = consts.tile([128, 8], BF16)
            ones8 = consts.tile([1, NB], BF16)
            ident8 = consts.tile([8, 8], F32)
            ident8b = consts.tile([8, 8], BF16)
            dec_rows = consts.tile([NB, N], BF16)
            decT = consts.tile([128, 8, NB], BF16)
            # softmax group j lives at partition base 32*j (32-aligned for
            # engine ops); packing 4 groups per tile costs 1x, not 4x, SBUF
            mask_p = consts.tile([98, T], F32)
            covf_p = consts.tile([98, T], F32)
            score_p = consts.tile([98, T], F32)
            attn_p = consts.tile([98, T], F32)
            covn_p = consts.tile([98, T], F32)
            ssum_p = consts.tile([98, 1], F32)
            rs_p = consts.tile([98, 1], F32)
            scrow = [consts.tile([1, T], F32, name=f"sr{i}") for i in range(2)]
            attn8_t = [consts.tile([8, 128], F32, name=f"a8{i}") for i in range(2)]
            acw = [consts.tile([128, 8, NB], BF16, name=f"acw{b}") for b in range(NB)]
            ct_sb = consts.tile([NB, N], F32)

            # ---------------- PSUM tiles ----------------
            att_ps = [psA.tile([128, N], F32, name=f"att{i}") for i in range(2)]
            shared = psS.tile([98, N], F32)      # ct rows 0-7, score @32/@64
            acp_f = psT.tile([128, 8], F32)      # attn transpose scratch
            acp_b = psT.tile([128, 8], BF16)     # dec transpose scratch

            # ---------------- startup DMAs ----------------
            # wpairs split across sync/gpsimd so dec can start ~2us in;
            # EF(0)/EF(1) ride the otherwise-empty vector queue
            nc.sync.dma_start(out=sT_all, in_=st_d.rearrange("(c p) b -> p c b", p=128))
            for c2 in range(4):
                q = nc.sync if c2 % 2 == 0 else nc.gpsimd
                q.dma_start(
                    out=wpairs[c2],
                    in_=wdt_d[c2, :, :].rearrange("p (k n) -> p k n", k=2),
                )
            nc.sync.dma_start(out=bd_b, in_=bd_d[None, :])
            nc.sync.dma_start(out=vt_sb, in_=vt_d[:, :])
            nc.gpsimd.dma_start(out=lhs_t, in_=lhs_d[:, :, :])
            for j in range(4):
                nc.gpsimd.dma_start(
                    out=mask_p[32 * j:32 * j + 2, :], in_=mk_d[2 * j:2 * j + 2, :]
                )
                nc.gpsimd.dma_start(
                    out=covf_p[32 * j:32 * j + 2, :], in_=cv_d[2 * j:2 * j + 2, :]
                )

            # identity + constants
            make_identity(nc, ident8)
            nc.vector.tensor_copy(ident8b, ident8)
            nc.vector.memset(ones8, 1.0)
            for i in range(3):
                nc.vector.memset(ef_bufs[i][:, 8, :], 0.0)
            for b in range(NB):
                nc.vector.memset(acw[b], 0.0)

            # ---------------- DMA helpers ----------------
            def load_ef(b):
                buf = ef_bufs[b % 3]
                nc.sync.dma_start(
                    out=buf[:, 0:8, :],
                    in_=ef_d[b, :, :, :].rearrange("c p t -> p c t"),
                )
                nc.gpsimd.dma_start(out=buf[0:1, 8, :], in_=cv8_d[b:b + 1, :])

            def load_eo(b):
                buf = eo_bufs[b % 4]
                nc.sync.dma_start(
                    out=buf, in_=eo_d[b, :, :, :].rearrange("c p n -> p c n")
                )

            # ---------------- compute emitters ----------------
            def emit_att(b, c):
                buf = ef_bufs[b % 3]
                ps = att_ps[c % 2]
                lhs_ap = lhs_t[:, 0:c + 2:(c + 1), :]
                for h in range(2):
                    nc.tensor.matmul(
                        ps[:, h * 512:(h + 1) * 512],
                        lhsT=lhs_ap,
                        rhs=buf[:, c:9:(8 - c), h * 512:(h + 1) * 512],
                        perf_mode=DR, start=True, stop=True,
                        skip_group_check=True,
                    )

            def emit_tanh(b, c):
                nc.scalar.activation(
                    th_bufs[c % 4], att_ps[c % 2], AF.Tanh,
                    bias=decT[:, c, b:b + 1],
                )

            def score_reg(b):
                return shared[32:33, :] if b % 2 == 0 else shared[64:65, :]

            def emit_score(b, c):
                th = th_bufs[c % 4]
                reg = score_reg(b)
                for h in range(2):
                    nc.tensor.matmul(
                        reg[:, h * 512:(h + 1) * 512],
                        lhsT=vt_sb[:, c:c + 1],
                        rhs=th[:, h * 512:(h + 1) * 512],
                        start=(c == 0), stop=(c == 7),
                        skip_group_check=True,
                    )

            def emit_score_drain(b):
                row = scrow[b % 2]
                nc.vector.tensor_copy(row, score_reg(b))
                p = 32 * (b // 2) + b % 2
                nc.gpsimd.dma_start(out=score_p[p:p + 1, :], in_=row)

            def emit_softmax(j):
                g = slice(32 * j, 32 * j + 2)
                sg, ag, vg = score_p[g, :], attn_p[g, :], covn_p[g, :]
                nc.scalar.activation(ag, sg, AF.Exp)
                nc.vector.scalar_tensor_tensor(
                    out=ag, in0=ag, scalar=1.0, in1=mask_p[g, :],
                    op0=ALU.mult, op1=ALU.mult, accum_out=ssum_p[g, :],
                )
                nc.vector.reciprocal(rs_p[g, :], ssum_p[g, :])
                nc.vector.tensor_scalar_mul(ag, ag, rs_p[g, :])
                nc.vector.tensor_add(vg, covf_p[g, :], ag)
                nc.gpsimd.dma_start(out=at_o[2 * j:2 * j + 2, :], in_=ag)
                nc.gpsimd.dma_start(out=cn_o[2 * j:2 * j + 2, :], in_=vg)

            def emit_acw(b):
                p = 32 * (b // 2) + b % 2
                a8 = attn8_t[b % 2]
                nc.gpsimd.dma_start(
                    out=a8,
                    in_=attn_p[p:p + 1, :].rearrange("p (c t) -> p c t", c=8),
                )
                nc.tensor.matmul(
                    acp_f, lhsT=a8, rhs=ident8[0:8, 0:8],
                    is_transpose=True, start=True, stop=True,
                )
                nc.vector.tensor_copy(acw[b][:, :, b], acp_f)

            def emit_ct(b, c):
                buf = eo_bufs[b % 4]
                for h in range(2):
                    nc.tensor.matmul(
                        shared[0:8, h * 512:(h + 1) * 512],
                        lhsT=acw[b][:, c, :],
                        rhs=buf[:, c, h * 512:(h + 1) * 512],
                        start=(b == 0 and c == 0),
                        stop=(b == NB - 1 and c == 7),
                        skip_group_check=True,
                    )

            def emit_dec():
                dv = att_ps[0]  # borrow before att(0, 0) overwrites it
                for kj in range(NCH):
                    for h in range(2):
                        nc.tensor.matmul(
                            dv[0:8, h * 512:(h + 1) * 512],
                            lhsT=sT_all[:, kj, :],
                            rhs=wpairs[kj // 2][:, kj % 2, h * 512:(h + 1) * 512],
                            start=(kj == 0), stop=False, skip_group_check=True,
                        )
                for h in range(2):
                    nc.tensor.matmul(
                        dv[0:8, h * 512:(h + 1) * 512],
                        lhsT=ones8, rhs=bd_b[0:1, h * 512:(h + 1) * 512],
                        start=False, stop=True, skip_group_check=True,
                    )
                nc.vector.tensor_copy(dec_rows, dv[0:8, :])
                for c in range(8):
                    nc.tensor.matmul(
                        acp_b, lhsT=dec_rows[:, c * 128:(c + 1) * 128],
                        rhs=ident8b[0:8, 0:8],
                        is_transpose=True, start=True, stop=True,
                    )
                    nc.vector.tensor_copy(decT[:, c, :], acp_b)

            # ---------------- schedule ----------------
            load_ef(0)
            load_ef(1)
            emit_dec()

            for k in range(11):
                if k + 2 <= NB - 1:
                    load_ef(k + 2)
                if k <= NB - 1:
                    load_eo(k)
                for c in range(8):
                    if c == 0 and 0 <= k - 2 <= 7 and (k - 2) % 2 == 1:
                        emit_acw(k - 2)
                    if k <= 7:
                        emit_att(k, c)
                        emit_tanh(k, c)
                    # score: slots 0-1 finish batch k-1, slots 2-7 run batch k
                    if c < 2:
                        sb_, sc_ = k - 1, 6 + c
                    else:
                        sb_, sc_ = k, c - 2
                    if 0 <= sb_ <= 7 and sb_ <= 7 and (k <= 7 or c < 2):
                        emit_score(sb_, sc_)
                    if c == 1 and 1 <= k <= 8:
                        emit_score_drain(k - 1)
                        if (k - 1) % 2 == 1:
                            emit_softmax((k - 1) // 2)
                            emit_acw(k - 2)
                    if 0 <= k - 3 <= 7:
                        emit_ct(k - 3, c)

            nc.vector.tensor_copy(ct_sb, shared[0:8, :])
            nc.sync.dma_start(out=ct_o[:, :], in_=ct_sb)

    nc.finalize()
    return nc


_CACHE: dict = {}


def _get_nc() -> bass.Bass:
    if "nc" not in _CACHE:
        _CACHE["nc"] = build_bass(NB)
    return _CACHE["nc"]


def make_in_maps(inputs: dict) -> list:
    f = lambda x: np.ascontiguousarray(np.asarray(x), dtype=np.float32)
    s = f(inputs["s_t_hat"])
    eo = f(inputs["encoder_outputs"])
    ef = f(inputs["encoder_feature"]).reshape(B, T, N)
    mk = f(inputs["enc_padding_mask"])
    cv = f(inputs["coverage"])
    wdt = np.ascontiguousarray(f(inputs["W_d"]).T).astype(ml_dtypes.bfloat16)
    wdt_pairs = np.ascontiguousarray(
        wdt.reshape(8, 128, N).transpose(1, 0, 2).reshape(128, 4, 2048)
        .transpose(1, 0, 2)
    )
    bd = f(inputs["b_d"]).astype(ml_dtypes.bfloat16)
    wc = f(inputs["W_c"])
    vv = f(inputs["v"])
    vt = np.ascontiguousarray(vv.reshape(8, 128).T).astype(ml_dtypes.bfloat16)

    lhs = np.zeros((128, 9, 128), np.float32)
    lhs[:, 0, :] = np.eye(128, dtype=np.float32)
    for c in range(8):
        lhs[0, c + 1, :] = wc[c * 128:(c + 1) * 128]
    lhs_blk = lhs.astype(ml_dtypes.float8_e4m3)

    in_maps = []
    for i in range(N_CORES):
        sl = slice(i * NB, (i + 1) * NB)
        # EF^T blocks: [b, c, p, t] = EF[b, t, c*128+p]
        ef_blk = np.ascontiguousarray(
            ef[sl].transpose(0, 2, 1).reshape(NB, NCH, 128, T)
        ).astype(ml_dtypes.float8_e4m3)
        eo_blk = np.ascontiguousarray(
            eo[sl].reshape(NB, 8, 128, N)
        ).astype(ml_dtypes.bfloat16)
        in_maps.append({
            "ef_blk": ef_blk,
            "cov_fp8": cv[sl].astype(ml_dtypes.float8_e4m3),
            "lhs_blk": lhs_blk,
            "eo_blk": eo_blk,
            "enc_padding_mask": mk[sl],
            "coverage": cv[sl],
            "W_d_T": wdt_pairs,
            "s_t_hat_T": np.ascontiguousarray(s[sl].T).astype(ml_dtypes.bfloat16),
            "b_d": bd,
            "v_T": vt,
        })
    return in_maps


def gather_outputs(results: list):
    c_t = np.concatenate([results[i]["c_t"] for i in range(N_CORES)], axis=0)
    attn = np.concatenate([results[i]["attn"] for i in range(N_CORES)], axis=0)
    covn = np.concatenate(
        [results[i]["coverage_next"] for i in range(N_CORES)], axis=0
    )
    return c_t, attn, covn


def kernel(**inputs):
    nc = _get_nc()
    in_maps = make_in_maps(inputs)
    res = run_bass_kernel_spmd(nc, in_maps, core_ids=list(range(N_CORES)))
    return gather_outputs(res.results)


# revision 3
# speedup vs baseline: 1.0236x; 1.0084x over previous
"""Trainium2 Bass kernel v3 for pointer-generator additive attention.

v2 (transposed pass-1, PE score-reduce) reached HW correctness but ran at
~188us: the PE never held its 2.4 GHz p-state because att(k,c) WAR-stalled
on tanh(k,c-2) every slot (2 PSUM att buffers) and score(k,c-2) waits were
satisfied just-in-time.  A back-to-back microbench shows the PE sustains
218 ns per 512-col matmul (full clock) and fp8 DoubleRow streams TWO
k-tiles in that same 218 ns, with LDWEIGHTS fully hidden.

v3 keeps v2's math but rebuilds the pipeline for PE continuity:
  - 3 att PSUM buffers [128,1024] (6 banks) -- att(k,c) now WARs on
    tanh(k,c-3), finished ~2 slots earlier.  The PSUM transpose scratch
    that blocked the 3rd buffer is gone:
      * attn one-hot columns come from a DMA xbar transpose
        ([16,128] bf16 -> [128,16]; aT[p,c] = attn[c*128+p]), fed by a
        bf16 copy of attn produced inside softmax.
      * dec transposes go through a borrowed att_ps[2] column region
        at startup, drained by ONE [128,64] DVE copy into decT.
  - score lag deepened to 4 slots; th ring = 6 bufs.
  - c_t lag 3 for batches 0-5; {6,7} get singleton softmaxes (batch 6 at
    packed row 96, batch 7 in standalone tiles) so ct(6) runs step 8 and
    ct(7) step 9 -- tail ~12us instead of ~34us.
  - ACT table preloaded via a dummy tanh at t=0.
  - shared PSUM [98,1024]: c_t rows 0-7, score rows 32/64 by parity.
"""

import sys

if "/opt/trn_rl_repo" not in sys.path:
    sys.path.insert(0, "/opt/trn_rl_repo")

import ml_dtypes
import numpy as np

import concourse.bass as bass
import concourse.mybir as mybir
import concourse.tile as tile
from concourse import bacc
from concourse.bass_utils import run_bass_kernel_spmd
from concourse.masks import make_identity

F32 = mybir.dt.float32
BF16 = mybir.dt.bfloat16
FP8 = mybir.dt.float8e4
AF = mybir.ActivationFunctionType
ALU = mybir.AluOpType
DR = mybir.MatmulPerfMode.DoubleRow

CUR = [""]   # emission label, for schedule debugging

N_CORES = 8
B = 64
NB = B // N_CORES  # 8 local batches per core
T = 1024
N = 1024
NCH = 8            # n-chunks of 128 per batch


def build_bass(nb: int = NB) -> bass.Bass:
    nc = bacc.Bacc()

    ef_d = nc.declare_dram_parameter("ef_blk", [nb, 9, 128, T], FP8, isOutput=False)
    lhs_d = nc.declare_dram_parameter("lhs_blk", [128, 9, 128], FP8, isOutput=False)
    eo_d = nc.declare_dram_parameter("eo_blk", [nb, 8, 128, N], BF16, isOutput=False)
    mk_d = nc.declare_dram_parameter("enc_padding_mask", [nb, T], F32, isOutput=False)
    cv_d = nc.declare_dram_parameter("coverage", [nb, T], F32, isOutput=False)
    wdt_d = nc.declare_dram_parameter("W_d_T", [4, 128, 2048], BF16, isOutput=False)
    st_d = nc.declare_dram_parameter("s_t_hat_T", [N, nb], BF16, isOutput=False)
    bd_d = nc.declare_dram_parameter("b_d", [N], BF16, isOutput=False)
    vt_d = nc.declare_dram_parameter("v_T", [128, 8], BF16, isOutput=False)
    id_d = nc.declare_dram_parameter("ident8", [8, 8], F32, isOutput=False)
    ct_o = nc.declare_dram_parameter("c_t", [nb, N], F32, isOutput=True)
    at_o = nc.declare_dram_parameter("attn", [nb, T], F32, isOutput=True)
    cn_o = nc.declare_dram_parameter("coverage_next", [nb, T], F32, isOutput=True)

    with tile.TileContext(nc) as tc:
        with (
            tc.tile_pool(name="consts", bufs=1) as consts,
            tc.tile_pool(name="efp", bufs=1) as efp,
            tc.tile_pool(name="eop", bufs=1) as eop,
            tc.tile_pool(name="thp", bufs=1) as thp,
            tc.tile_pool(name="psA", bufs=1, space="PSUM") as psA,
            tc.tile_pool(name="psS", bufs=1, space="PSUM") as psS,
        ):
            # ---------------- SBUF tiles ----------------
            ef_bufs = [efp.tile([128, 9, T], FP8, name=f"ef{i}") for i in range(3)]
            eo_bufs = [eop.tile([128, 8, N], BF16, name=f"eo{i}") for i in range(4)]
            th_bufs = [thp.tile([128, N], BF16, name=f"th{i}") for i in range(6)]

            lhs_t = consts.tile([128, 9, 128], FP8)
            sT_all = consts.tile([128, NCH, NB], BF16)
            wpairs = [consts.tile([128, 2, N], BF16, name=f"wp{i}") for i in range(4)]
            bd_b = consts.tile([1, N], BF16)
            vt_sb = consts.tile([128, 8], BF16)
            ones8 = consts.tile([1, NB], BF16)
            ident8 = consts.tile([8, 8], F32)
            dec_rows = consts.tile([NB, N], F32)
            decT = consts.tile([128, 8, NB], BF16)
            dummy = consts.tile([1, 1], BF16)
            # softmax groups: {0,1}@0 {2,3}@32 {4,5}@64 {6}@96, batch 7 standalone
            mask_p = consts.tile([98, T], F32)
            covf_p = consts.tile([98, T], F32)
            score_p = consts.tile([98, T], F32)
            attn_p = consts.tile([98, T], F32)
            attnb_p = consts.tile([98, T], BF16)
            covn_p = consts.tile([98, T], F32)
            ssum_p = consts.tile([98, 1], F32)
            rs_p = consts.tile([98, 1], F32)
            mask_7 = consts.tile([1, T], F32)
            covf_7 = consts.tile([1, T], F32)
            score_7 = consts.tile([1, T], F32)
            attn_7 = consts.tile([1, T], F32)
            attnb_7 = consts.tile([1, T], BF16)
            covn_7 = consts.tile([1, T], F32)
            ssum_7 = consts.tile([1, 1], F32)
            rs_7 = consts.tile([1, 1], F32)
            scrow = [consts.tile([1, T], F32, name=f"sr{i}") for i in range(2)]
            a16 = [consts.tile([16, 128], BF16, name=f"a16_{i}") for i in range(2)]
            aT16 = [consts.tile([128, 16], BF16, name=f"aT16_{i}") for i in range(2)]
            acw = [consts.tile([128, 8, NB], BF16, name=f"acw{b}") for b in range(NB)]
            ct_sb = consts.tile([NB, N], F32)

            # ---------------- PSUM tiles: 6 + 2 = 8 banks ----------------
            att_ps = [psA.tile([128, N], F32, name=f"att{i}") for i in range(3)]
            shared = psS.tile([98, N], F32)   # ct rows 0-7, score @32 / @64

            # ---------------- startup DMAs ----------------
            nc.sync.dma_start(out=sT_all, in_=st_d.rearrange("(c p) b -> p c b", p=128))
            for c2 in range(4):
                q = nc.sync if c2 % 2 == 0 else nc.gpsimd
                q.dma_start(
                    out=wpairs[c2],
                    in_=wdt_d[c2, :, :].rearrange("p (k n) -> p k n", k=2),
                )
            nc.sync.dma_start(out=bd_b, in_=bd_d[None, :])
            nc.sync.dma_start(out=vt_sb, in_=vt_d[:, :])
            nc.gpsimd.dma_start(out=lhs_t, in_=lhs_d[:, :, :])

            nc.sync.dma_start(out=ident8, in_=id_d[:, :])
            nc.vector.memset(ones8, 1.0)
            # ACT table preload off the critical path
            nc.scalar.activation(dummy, ones8[0:1, 0:1], AF.Tanh)
            for b in range(NB):
                nc.vector.memset(acw[b], 0.0)
            for i in range(2):
                nc.vector.memset(a16[i], 0.0)

            # ---------------- DMA helpers ----------------
            def load_ef(b):
                CUR[0] = f'EF{b}'
                buf = ef_bufs[b % 3]
                nc.sync.dma_start(
                    out=buf,
                    in_=ef_d[b, :, :, :].rearrange("c p t -> p c t"),
                )

            def load_eo(b):
                CUR[0] = f'EO{b}'
                buf = eo_bufs[b % 4]
                nc.sync.dma_start(
                    out=buf, in_=eo_d[b, :, :, :].rearrange("c p n -> p c n")
                )

            # ---------------- compute emitters ----------------
            def emit_att(b, c):
                CUR[0] = f'att{b}_{c}'
                buf = ef_bufs[b % 3]
                ps = att_ps[(8 * b + c) % 3]
                lhs_ap = lhs_t[:, 0:c + 2:(c + 1), :]
                for h in range(2):
                    nc.tensor.matmul(
                        ps[:, h * 512:(h + 1) * 512],
                        lhsT=lhs_ap,
                        rhs=buf[:, c:9:(8 - c), h * 512:(h + 1) * 512],
                        perf_mode=DR, start=True, stop=True,
                        skip_group_check=True,
                    )

            def emit_tanh(b, c):
                CUR[0] = f'tanh{b}_{c}'
                nc.scalar.activation(
                    th_bufs[(8 * b + c) % 6], att_ps[(8 * b + c) % 3], AF.Tanh,
                    bias=decT[:, c, b:b + 1],
                )

            def score_reg(b):
                return shared[32:33, :] if b % 2 == 0 else shared[64:65, :]

            def emit_score(b, c):
                CUR[0] = f'score{b}_{c}'
                th = th_bufs[(8 * b + c) % 6]
                reg = score_reg(b)
                for h in range(2):
                    nc.tensor.matmul(
                        reg[:, h * 512:(h + 1) * 512],
                        lhsT=vt_sb[:, c:c + 1],
                        rhs=th[:, h * 512:(h + 1) * 512],
                        start=(c == 0), stop=(c == 7),
                        skip_group_check=True,
                    )

            def sm_tiles(b):
                """(score, attn, attn_bf, covn, mask, covf, ssum, rs) APs and
                the group slice holding batch b."""
                if b == 7:
                    return (score_7, attn_7, attnb_7, covn_7, mask_7, covf_7,
                            ssum_7, rs_7, 0)
                base = 32 * (b // 2) if b < 6 else 96
                return (score_p, attn_p, attnb_p, covn_p, mask_p, covf_p,
                        ssum_p, rs_p, base)

            def emit_score_drain(b):
                CUR[0] = f'drain{b}'
                row = scrow[b % 2]
                nc.vector.tensor_copy(row, score_reg(b))
                sg, _, _, _, _, _, _, _, base = sm_tiles(b)
                r = base + (b % 2 if b < 6 else 0)
                nc.gpsimd.dma_start(out=sg[r:r + 1, :], in_=row)

            def emit_softmax(lo_b, nr):
                CUR[0] = f'softmax{lo_b}'
                sg, ag, ab, vg, mg, cg, ss, rs, base = sm_tiles(lo_b)
                g = slice(base, base + nr)
                nc.scalar.activation(ag[g, :], sg[g, :], AF.Exp)
                nc.vector.scalar_tensor_tensor(
                    out=ag[g, :], in0=ag[g, :], scalar=1.0, in1=mg[g, :],
                    op0=ALU.mult, op1=ALU.mult, accum_out=ss[g, :],
                )
                nc.vector.reciprocal(rs[g, :], ss[g, :])
                nc.vector.tensor_scalar_mul(ag[g, :], ag[g, :], rs[g, :])
                nc.vector.tensor_copy(ab[g, :], ag[g, :])
                nc.vector.tensor_add(vg[g, :], cg[g, :], ag[g, :])
                nc.gpsimd.dma_start(out=at_o[lo_b:lo_b + nr, :], in_=ag[g, :])
                nc.gpsimd.dma_start(out=cn_o[lo_b:lo_b + nr, :], in_=vg[g, :])

            def emit_acw(b):
                CUR[0] = f'acw{b}'
                _, _, ab, _, _, _, _, _, base = sm_tiles(b)
                r = base + (b % 2 if b < 6 else 0)
                t16 = a16[b % 2]
                nc.gpsimd.dma_start(
                    out=t16[0:8, :],
                    in_=ab[r:r + 1, :].rearrange("p (c t) -> p c t", c=8),
                )
                aT = aT16[b % 2]
                nc.scalar.dma_start_transpose(out=aT, in_=t16)
                nc.vector.tensor_copy(acw[b][:, :, b], aT[:, 0:8])

            def emit_ct(b, c):
                CUR[0] = f'ct{b}_{c}'
                buf = eo_bufs[b % 4]
                for h in range(2):
                    nc.tensor.matmul(
                        shared[0:8, h * 512:(h + 1) * 512],
                        lhsT=acw[b][:, c, :],
                        rhs=buf[:, c, h * 512:(h + 1) * 512],
                        start=(b == 0 and c == 0),
                        stop=(b == NB - 1 and c == 7),
                        skip_group_check=True,
                    )

            def emit_dec():
                CUR[0] = 'dec'
                dv = att_ps[1]   # matvec accumulator (rows 0-7)
                sc = att_ps[2]   # transpose scratch (cols 0-63)
                for kj in range(NCH):
                    for h in range(2):
                        nc.tensor.matmul(
                            dv[0:8, h * 512:(h + 1) * 512],
                            lhsT=sT_all[:, kj, :],
                            rhs=wpairs[kj // 2][:, kj % 2, h * 512:(h + 1) * 512],
                            start=(kj == 0), stop=False, skip_group_check=True,
                        )
                for h in range(2):
                    nc.tensor.matmul(
                        dv[0:8, h * 512:(h + 1) * 512],
                        lhsT=ones8, rhs=bd_b[0:1, h * 512:(h + 1) * 512],
                        start=False, stop=True, skip_group_check=True,
                    )
                nc.vector.tensor_copy(dec_rows, dv[0:8, :])
                for c in range(8):
                    nc.tensor.matmul(
                        sc[:, c * 8:(c + 1) * 8],
                        lhsT=dec_rows[:, c * 128:(c + 1) * 128],
                        rhs=ident8[0:8, 0:8],
                        is_transpose=True, start=True, stop=True,
                        skip_group_check=True,
                    )
                nc.vector.tensor_copy(
                    decT.rearrange("p c b -> p (c b)"), sc[:, 0:64]
                )

            # ---------------- schedule ----------------
            load_ef(0)
            load_ef(1)
            emit_att(0, 0)
            emit_dec()

            for k in range(10):
                for c in range(8):
                    if k <= 7 and not (k == 0 and c == 0):
                        emit_att(k, c)
                    if k <= 7:
                        emit_tanh(k, c)
                    # score lag 4: slots 0-3 finish batch k-1, 4-7 run batch k
                    if c < 4:
                        sb_, sc_ = k - 1, 4 + c
                    else:
                        sb_, sc_ = k, c - 4
                    if 0 <= sb_ <= 7 and (sb_ == k - 1 or k <= 7):
                        emit_score(sb_, sc_)
                    if c == 3 and 1 <= k <= 8:
                        emit_score_drain(k - 1)
                    if c == 5 and 1 <= k <= 8:
                        kb = k - 1  # last batch whose score just drained
                        if kb % 2 == 1 and kb <= 5:
                            emit_softmax(kb - 1, 2)
                            emit_acw(kb - 1)
                            emit_acw(kb)
                        elif kb >= 6:
                            emit_softmax(kb, 1)
                            emit_acw(kb)
                    # c_t: batches 0-5 at lag 3; 6 at step 8; 7 at step 9
                    if 3 <= k <= 8 and k - 3 <= 5:
                        emit_ct(k - 3, c)
                    if k == 8:
                        emit_ct(6, c)
                    if k == 9:
                        emit_ct(7, c)
                # bulk loads at end of step: latency-critical smalls (score
                # gather, acw xbar) issued mid-step go ahead of them in the
                # sync queue; prefetch depth still covers arrival
                if k + 2 <= NB - 1:
                    load_ef(k + 2)
                if k <= NB - 1:
                    load_eo(k)
                if k == 0:
                    for j in range(3):
                        nc.sync.dma_start(
                            out=mask_p[32 * j:32 * j + 2, :],
                            in_=mk_d[2 * j:2 * j + 2, :],
                        )
                        nc.sync.dma_start(
                            out=covf_p[32 * j:32 * j + 2, :],
                            in_=cv_d[2 * j:2 * j + 2, :],
                        )
                    nc.sync.dma_start(out=mask_p[96:97, :], in_=mk_d[6:7, :])
                    nc.sync.dma_start(out=covf_p[96:97, :], in_=cv_d[6:7, :])
                    nc.sync.dma_start(out=mask_7, in_=mk_d[7:8, :])
                    nc.sync.dma_start(out=covf_7, in_=cv_d[7:8, :])

            nc.vector.tensor_copy(ct_sb, shared[0:8, :])
            nc.sync.dma_start(out=ct_o[:, :], in_=ct_sb)

    nc.finalize()
    return nc


_CACHE: dict = {}


def _get_nc() -> bass.Bass:
    if "nc" not in _CACHE:
        _CACHE["nc"] = build_bass(NB)
    return _CACHE["nc"]


def make_in_maps(inputs: dict) -> list:
    f = lambda x: np.ascontiguousarray(np.asarray(x), dtype=np.float32)
    s = f(inputs["s_t_hat"])
    eo = f(inputs["encoder_outputs"])
    ef = f(inputs["encoder_feature"]).reshape(B, T, N)
    mk = f(inputs["enc_padding_mask"])
    cv = f(inputs["coverage"])
    wdt = np.ascontiguousarray(f(inputs["W_d"]).T).astype(ml_dtypes.bfloat16)
    wdt_pairs = np.ascontiguousarray(
        wdt.reshape(8, 128, N).transpose(1, 0, 2).reshape(128, 4, 2048)
        .transpose(1, 0, 2)
    )
    bd = f(inputs["b_d"]).astype(ml_dtypes.bfloat16)
    wc = f(inputs["W_c"])
    vv = f(inputs["v"])
    vt = np.ascontiguousarray(vv.reshape(8, 128).T).astype(ml_dtypes.bfloat16)

    lhs = np.zeros((128, 9, 128), np.float32)
    lhs[:, 0, :] = np.eye(128, dtype=np.float32)
    for c in range(8):
        lhs[0, c + 1, :] = wc[c * 128:(c + 1) * 128]
    lhs_blk = lhs.astype(ml_dtypes.float8_e4m3)

    in_maps = []
    for i in range(N_CORES):
        sl = slice(i * NB, (i + 1) * NB)
        ef_blk = np.zeros((NB, 9, 128, T), ml_dtypes.float8_e4m3)
        ef_blk[:, 0:8] = ef[sl].transpose(0, 2, 1).reshape(
            NB, NCH, 128, T).astype(ml_dtypes.float8_e4m3)
        ef_blk[:, 8, 0, :] = cv[sl].astype(ml_dtypes.float8_e4m3)
        eo_blk = np.ascontiguousarray(
            eo[sl].reshape(NB, 8, 128, N)
        ).astype(ml_dtypes.bfloat16)
        in_maps.append({
            "ef_blk": ef_blk,
            "lhs_blk": lhs_blk,
            "eo_blk": eo_blk,
            "enc_padding_mask": mk[sl],
            "coverage": cv[sl],
            "W_d_T": wdt_pairs,
            "s_t_hat_T": np.ascontiguousarray(s[sl].T).astype(ml_dtypes.bfloat16),
            "b_d": bd,
            "v_T": vt,
            "ident8": np.eye(8, dtype=np.float32),
        })
    return in_maps


def gather_outputs(results: list):
    c_t = np.concatenate([results[i]["c_t"] for i in range(N_CORES)], axis=0)
    attn = np.concatenate([results[i]["attn"] for i in range(N_CORES)], axis=0)
    covn = np.concatenate(
        [results[i]["coverage_next"] for i in range(N_CORES)], axis=0
    )
    return c_t, attn, covn


def kernel(**inputs):
    nc = _get_nc()
    in_maps = make_in_maps(inputs)
    res = run_bass_kernel_spmd(nc, in_maps, core_ids=list(range(N_CORES)))
    return gather_outputs(res.results)


# revision 4
# speedup vs baseline: 1.1741x; 1.1470x over previous
"""Trainium2 Bass kernel v3 for pointer-generator additive attention.

v2 (transposed pass-1, PE score-reduce) reached HW correctness but ran at
~188us: the PE never held its 2.4 GHz p-state because att(k,c) WAR-stalled
on tanh(k,c-2) every slot (2 PSUM att buffers) and score(k,c-2) waits were
satisfied just-in-time.  A back-to-back microbench shows the PE sustains
218 ns per 512-col matmul (full clock) and fp8 DoubleRow streams TWO
k-tiles in that same 218 ns, with LDWEIGHTS fully hidden.

v3 keeps v2's math but rebuilds the pipeline for PE continuity:
  - 3 att PSUM buffers [128,1024] (6 banks) -- att(k,c) now WARs on
    tanh(k,c-3), finished ~2 slots earlier.  The PSUM transpose scratch
    that blocked the 3rd buffer is gone:
      * attn one-hot columns come from a DMA xbar transpose
        ([16,128] bf16 -> [128,16]; aT[p,c] = attn[c*128+p]), fed by a
        bf16 copy of attn produced inside softmax.
      * dec transposes go through a borrowed att_ps[2] column region
        at startup, drained by ONE [128,64] DVE copy into decT.
  - score lag deepened to 4 slots; th ring = 6 bufs.
  - c_t lag 3 for batches 0-5; {6,7} get singleton softmaxes (batch 6 at
    packed row 96, batch 7 in standalone tiles) so ct(6) runs step 8 and
    ct(7) step 9 -- tail ~12us instead of ~34us.
  - ACT table preloaded via a dummy tanh at t=0.
  - shared PSUM [98,1024]: c_t rows 0-7, score rows 32/64 by parity.
"""

import sys

if "/opt/trn_rl_repo" not in sys.path:
    sys.path.insert(0, "/opt/trn_rl_repo")

import ml_dtypes
import numpy as np

import concourse.bass as bass
import concourse.mybir as mybir
import concourse.tile as tile
from concourse import bacc
from concourse.bass_utils import run_bass_kernel_spmd
from concourse.masks import make_identity

F32 = mybir.dt.float32
BF16 = mybir.dt.bfloat16
FP8 = mybir.dt.float8e4
AF = mybir.ActivationFunctionType
ALU = mybir.AluOpType
DR = mybir.MatmulPerfMode.DoubleRow

CUR = [""]   # emission label, for schedule debugging

N_CORES = 8
B = 64
NB = B // N_CORES  # 8 local batches per core
T = 1024
N = 1024
NCH = 8            # n-chunks of 128 per batch


def build_bass(nb: int = NB) -> bass.Bass:
    nc = bacc.Bacc()

    ef_d = nc.declare_dram_parameter("ef_blk", [nb, 9, 128, T], FP8, isOutput=False)
    lhs_d = nc.declare_dram_parameter("lhs_blk", [128, 9, 128], FP8, isOutput=False)
    eo_d = nc.declare_dram_parameter("eo_blk", [nb, 8, 128, N], BF16, isOutput=False)
    mk_d = nc.declare_dram_parameter("enc_padding_mask", [nb, T], F32, isOutput=False)
    cv_d = nc.declare_dram_parameter("coverage", [nb, T], F32, isOutput=False)
    wdt_d = nc.declare_dram_parameter("W_d_T", [4, 128, 2048], BF16, isOutput=False)
    st_d = nc.declare_dram_parameter("s_t_hat_T", [N, nb], BF16, isOutput=False)
    bd_d = nc.declare_dram_parameter("b_d", [N], BF16, isOutput=False)
    vt_d = nc.declare_dram_parameter("v_T", [128, 8], BF16, isOutput=False)
    id_d = nc.declare_dram_parameter("ident8", [8, 8], F32, isOutput=False)
    ct_o = nc.declare_dram_parameter("c_t", [nb, N], F32, isOutput=True)
    at_o = nc.declare_dram_parameter("attn", [nb, T], F32, isOutput=True)
    cn_o = nc.declare_dram_parameter("coverage_next", [nb, T], F32, isOutput=True)

    with tile.TileContext(nc) as tc:
        with (
            tc.tile_pool(name="consts", bufs=1) as consts,
            tc.tile_pool(name="efp", bufs=1) as efp,
            tc.tile_pool(name="eop", bufs=1) as eop,
            tc.tile_pool(name="thp", bufs=1) as thp,
            tc.tile_pool(name="psA", bufs=1, space="PSUM") as psA,
            tc.tile_pool(name="psS", bufs=1, space="PSUM") as psS,
        ):
            # ---------------- SBUF tiles ----------------
            ef_bufs = [efp.tile([128, 9, T], FP8, name=f"ef{i}") for i in range(3)]
            eo_bufs = [eop.tile([128, 8, N], BF16, name=f"eo{i}") for i in range(5)]
            th_bufs = [thp.tile([128, N], BF16, name=f"th{i}") for i in range(6)]

            lhs_t = consts.tile([128, 9, 128], FP8)
            sT_all = consts.tile([128, NCH, NB], BF16)
            wpairs = [consts.tile([128, 2, N], BF16, name=f"wp{i}") for i in range(4)]
            bd_b = consts.tile([1, N], BF16)
            vt_sb = consts.tile([128, 8], BF16)
            ones8 = consts.tile([1, NB], BF16)
            ident8 = consts.tile([8, 8], F32)
            dec_rows = consts.tile([NB, N], F32)
            decT = consts.tile([128, 8, NB], BF16)
            dummy = consts.tile([1, 1], BF16)
            # singleton softmax: batch b lives at row 32*(b%4) of set b//4
            # (32-aligned partition bases for engine ops)
            mask_s = [consts.tile([98, T], F32, name=f"mk{i}") for i in range(2)]
            covf_s = [consts.tile([98, T], F32, name=f"cf{i}") for i in range(2)]
            attn_s = [consts.tile([98, T], F32, name=f"at{i}") for i in range(2)]
            attnb_s = [consts.tile([98, T], BF16, name=f"ab{i}") for i in range(2)]
            covn_s = [consts.tile([98, T], F32, name=f"cn{i}") for i in range(2)]
            ssum_s = [consts.tile([98, 1], F32, name=f"ss{i}") for i in range(2)]
            rs_s = [consts.tile([98, 1], F32, name=f"rs{i}") for i in range(2)]
            a16 = [consts.tile([16, 128], BF16, name=f"a16_{i}") for i in range(2)]
            aT16 = [consts.tile([128, 16], BF16, name=f"aT16_{i}") for i in range(2)]
            acw = [consts.tile([128, 8, NB], BF16, name=f"acw{b}") for b in range(NB)]
            ct_sb = consts.tile([NB, N], F32)

            # ---------------- PSUM tiles: 6 + 2 = 8 banks ----------------
            att_ps = [psA.tile([128, N], F32, name=f"att{i}") for i in range(3)]
            shared = psS.tile([98, N], F32)   # ct rows 0-7, score @32 / @64

            # ---------------- startup DMAs ----------------
            nc.sync.dma_start(out=sT_all, in_=st_d.rearrange("(c p) b -> p c b", p=128))
            for c2 in range(4):
                q = nc.sync if c2 % 2 == 0 else nc.gpsimd
                q.dma_start(
                    out=wpairs[c2],
                    in_=wdt_d[c2, :, :].rearrange("p (k n) -> p k n", k=2),
                )
            nc.sync.dma_start(out=bd_b, in_=bd_d[None, :])
            nc.sync.dma_start(out=vt_sb, in_=vt_d[:, :])
            nc.gpsimd.dma_start(out=lhs_t, in_=lhs_d[:, :, :])

            nc.sync.dma_start(out=ident8, in_=id_d[:, :])
            for b in range(4):
                nc.gpsimd.dma_start(
                    out=mask_s[0][32 * b:32 * b + 1, :], in_=mk_d[b:b + 1, :]
                )
                nc.gpsimd.dma_start(
                    out=covf_s[0][32 * b:32 * b + 1, :], in_=cv_d[b:b + 1, :]
                )
            nc.vector.memset(ones8, 1.0)
            # ACT table preload off the critical path
            nc.scalar.activation(dummy, ones8[0:1, 0:1], AF.Tanh)
            for b in range(NB):
                nc.vector.memset(acw[b], 0.0)
            for i in range(2):
                nc.vector.memset(a16[i], 0.0)

            # ---------------- DMA helpers ----------------
            def load_ef(b):
                CUR[0] = f'EF{b}'
                buf = ef_bufs[b % 3]
                nc.sync.dma_start(
                    out=buf,
                    in_=ef_d[b, :, :, :].rearrange("c p t -> p c t"),
                )

            def load_eo(b):
                CUR[0] = f'EO{b}'
                buf = eo_bufs[b % 5]
                nc.sync.dma_start(
                    out=buf, in_=eo_d[b, :, :, :].rearrange("c p n -> p c n")
                )

            # ---------------- compute emitters ----------------
            def emit_att(b, c):
                CUR[0] = f'att{b}_{c}'
                buf = ef_bufs[b % 3]
                ps = att_ps[(8 * b + c) % 3]
                lhs_ap = lhs_t[:, 0:c + 2:(c + 1), :]
                for h in range(2):
                    nc.tensor.matmul(
                        ps[:, h * 512:(h + 1) * 512],
                        lhsT=lhs_ap,
                        rhs=buf[:, c:9:(8 - c), h * 512:(h + 1) * 512],
                        perf_mode=DR, start=True, stop=True,
                        skip_group_check=True,
                    )

            def emit_tanh(b, c):
                CUR[0] = f'tanh{b}_{c}'
                nc.scalar.activation(
                    th_bufs[(8 * b + c) % 6], att_ps[(8 * b + c) % 3], AF.Tanh,
                    bias=decT[:, c, b:b + 1],
                )

            def score_reg(b):
                return shared[32:33, :] if b % 2 == 0 else shared[64:65, :]

            def emit_score(b, c):
                CUR[0] = f'score{b}_{c}'
                th = th_bufs[(8 * b + c) % 6]
                reg = score_reg(b)
                for h in range(2):
                    nc.tensor.matmul(
                        reg[:, h * 512:(h + 1) * 512],
                        lhsT=vt_sb[:, c:c + 1],
                        rhs=th[:, h * 512:(h + 1) * 512],
                        start=(c == 0), stop=(c == 7),
                        skip_group_check=True,
                    )

            def emit_softmax(b):
                CUR[0] = f'softmax{b}'
                i, g = b // 4, slice(32 * (b % 4), 32 * (b % 4) + 1)
                ag, ab = attn_s[i][g, :], attnb_s[i][g, :]
                # exp straight from the score PSUM region: no drain, no gather
                nc.scalar.activation(ag, score_reg(b), AF.Exp)
                nc.vector.scalar_tensor_tensor(
                    out=ag, in0=ag, scalar=1.0, in1=mask_s[i][g, :],
                    op0=ALU.mult, op1=ALU.mult, accum_out=ssum_s[i][g, :],
                )
                nc.vector.reciprocal(rs_s[i][g, :], ssum_s[i][g, :])
                nc.vector.tensor_scalar_mul(ag, ag, rs_s[i][g, :])
                nc.vector.tensor_copy(ab, ag)
                nc.vector.tensor_add(
                    covn_s[i][g, :], covf_s[i][g, :], ag
                )
                nc.gpsimd.dma_start(out=at_o[b:b + 1, :], in_=ag)
                nc.gpsimd.dma_start(out=cn_o[b:b + 1, :], in_=covn_s[i][g, :])

            def emit_acw(b):
                CUR[0] = f'acw{b}'
                i, r = b // 4, 32 * (b % 4)
                t16 = a16[b % 2]
                # regroup + xbar both on sync, issued ahead of the step's bulk
                nc.sync.dma_start(
                    out=t16[0:8, :],
                    in_=attnb_s[i][r:r + 1, :].rearrange("p (c t) -> p c t", c=8),
                )
                aT = aT16[b % 2]
                nc.sync.dma_start_transpose(out=aT, in_=t16)
                nc.vector.tensor_copy(acw[b][:, :, b], aT[:, 0:8])

            def emit_ct(b, c):
                CUR[0] = f'ct{b}_{c}'
                buf = eo_bufs[b % 5]
                for h in range(2):
                    nc.tensor.matmul(
                        shared[0:8, h * 512:(h + 1) * 512],
                        lhsT=acw[b][:, c, :],
                        rhs=buf[:, c, h * 512:(h + 1) * 512],
                        start=(b == 0 and c == 0),
                        stop=(b == NB - 1 and c == 7),
                        skip_group_check=True,
                    )

            def emit_dec():
                CUR[0] = 'dec'
                dv = att_ps[1]   # matvec accumulator (rows 0-7)
                sc = att_ps[2]   # transpose scratch (cols 0-63)
                dT = decT.rearrange("p c b -> p (c b)")
                # h-split: n-halves pipelined so tanh(0,0) starts ~4us sooner
                for h in range(2):
                    for kj in range(NCH):
                        nc.tensor.matmul(
                            dv[0:8, h * 512:(h + 1) * 512],
                            lhsT=sT_all[:, kj, :],
                            rhs=wpairs[kj // 2][:, kj % 2, h * 512:(h + 1) * 512],
                            start=(kj == 0), stop=False, skip_group_check=True,
                        )
                    nc.tensor.matmul(
                        dv[0:8, h * 512:(h + 1) * 512],
                        lhsT=ones8, rhs=bd_b[0:1, h * 512:(h + 1) * 512],
                        start=False, stop=True, skip_group_check=True,
                    )
                    nc.vector.tensor_copy(
                        dec_rows[:, h * 512:(h + 1) * 512],
                        dv[0:8, h * 512:(h + 1) * 512],
                    )
                    for c in range(4 * h, 4 * h + 4):
                        nc.tensor.matmul(
                            sc[:, c * 8:(c + 1) * 8],
                            lhsT=dec_rows[:, c * 128:(c + 1) * 128],
                            rhs=ident8[0:8, 0:8],
                            is_transpose=True, start=True, stop=True,
                            skip_group_check=True,
                        )
                    nc.vector.tensor_copy(
                        dT[:, 32 * h:32 * h + 32],
                        sc[:, 32 * h:32 * h + 32],
                    )

            # ---------------- schedule ----------------
            load_ef(0)
            load_ef(1)
            emit_att(0, 0)
            emit_dec()

            for k in range(10):
                for c in range(8):
                    if k <= 7 and not (k == 0 and c == 0):
                        emit_att(k, c)
                    if k <= 7:
                        emit_tanh(k, c)
                    # score lag 4: slots 0-3 finish batch k-1, 4-7 run batch
                    # k; batch 7 runs at lag 2 so the tail starts sooner
                    if c < 4:
                        sb_, sc_ = k - 1, 4 + c
                    else:
                        sb_, sc_ = k, c - 4
                    if 0 <= sb_ <= 6 and (sb_ == k - 1 or k <= 7):
                        emit_score(sb_, sc_)
                    if k == 7 and c >= 2:
                        emit_score(7, c - 2)
                    if k == 8 and c < 2:
                        emit_score(7, 6 + c)
                    if c == 3 and 1 <= k <= 7:
                        emit_softmax(k - 1)
                        emit_acw(k - 1)
                    if k == 8 and c == 2:
                        emit_softmax(7)
                        emit_acw(7)
                    # c_t: lag 4 (acw chain gets ~1.5 steps of slack);
                    # tail: ct(5), ct(6) at step 8, ct(7) at step 9
                    if 4 <= k <= 8 and k - 4 <= 4:
                        emit_ct(k - 4, c)
                    if k == 8:
                        emit_ct(5, c)
                        emit_ct(6, c)
                    if k == 9:
                        emit_ct(7, c)
                # bulk loads at end of step: latency-critical smalls (score
                # gather, acw xbar) issued mid-step go ahead of them in the
                # sync queue; prefetch depth still covers arrival
                if k + 2 <= NB - 1:
                    load_ef(k + 2)
                # EO shifted one step early: WAR on buf (k+1)%4 vs ct(k-3)
                # is emitted just above, and the tail then has no bulk DMA
                # blocking the acw smalls
                if k == 0:
                    load_eo(0)
                    load_eo(1)
                elif k + 1 <= NB - 1:
                    load_eo(k + 1)
                if k == 0:
                    for b in range(4, 8):
                        nc.gpsimd.dma_start(
                            out=mask_s[1][32 * (b % 4):32 * (b % 4) + 1, :],
                            in_=mk_d[b:b + 1, :],
                        )
                        nc.gpsimd.dma_start(
                            out=covf_s[1][32 * (b % 4):32 * (b % 4) + 1, :],
                            in_=cv_d[b:b + 1, :],
                        )

            nc.vector.tensor_copy(ct_sb, shared[0:8, :])
            nc.sync.dma_start(out=ct_o[:, :], in_=ct_sb)

    nc.finalize()
    return nc


_CACHE: dict = {}


def _get_nc() -> bass.Bass:
    if "nc" not in _CACHE:
        _CACHE["nc"] = build_bass(NB)
    return _CACHE["nc"]


def make_in_maps(inputs: dict) -> list:
    f = lambda x: np.ascontiguousarray(np.asarray(x), dtype=np.float32)
    s = f(inputs["s_t_hat"])
    eo = f(inputs["encoder_outputs"])
    ef = f(inputs["encoder_feature"]).reshape(B, T, N)
    mk = f(inputs["enc_padding_mask"])
    cv = f(inputs["coverage"])
    wdt = np.ascontiguousarray(f(inputs["W_d"]).T).astype(ml_dtypes.bfloat16)
    wdt_pairs = np.ascontiguousarray(
        wdt.reshape(8, 128, N).transpose(1, 0, 2).reshape(128, 4, 2048)
        .transpose(1, 0, 2)
    )
    bd = f(inputs["b_d"]).astype(ml_dtypes.bfloat16)
    wc = f(inputs["W_c"])
    vv = f(inputs["v"])
    vt = np.ascontiguousarray(vv.reshape(8, 128).T).astype(ml_dtypes.bfloat16)

    lhs = np.zeros((128, 9, 128), np.float32)
    lhs[:, 0, :] = np.eye(128, dtype=np.float32)
    for c in range(8):
        lhs[0, c + 1, :] = wc[c * 128:(c + 1) * 128]
    lhs_blk = lhs.astype(ml_dtypes.float8_e4m3)

    in_maps = []
    for i in range(N_CORES):
        sl = slice(i * NB, (i + 1) * NB)
        ef_blk = np.zeros((NB, 9, 128, T), ml_dtypes.float8_e4m3)
        ef_blk[:, 0:8] = ef[sl].transpose(0, 2, 1).reshape(
            NB, NCH, 128, T).astype(ml_dtypes.float8_e4m3)
        ef_blk[:, 8, 0, :] = cv[sl].astype(ml_dtypes.float8_e4m3)
        eo_blk = np.ascontiguousarray(
            eo[sl].reshape(NB, 8, 128, N)
        ).astype(ml_dtypes.bfloat16)
        in_maps.append({
            "ef_blk": ef_blk,
            "lhs_blk": lhs_blk,
            "eo_blk": eo_blk,
            "enc_padding_mask": mk[sl],
            "coverage": cv[sl],
            "W_d_T": wdt_pairs,
            "s_t_hat_T": np.ascontiguousarray(s[sl].T).astype(ml_dtypes.bfloat16),
            "b_d": bd,
            "v_T": vt,
            "ident8": np.eye(8, dtype=np.float32),
        })
    return in_maps


def gather_outputs(results: list):
    c_t = np.concatenate([results[i]["c_t"] for i in range(N_CORES)], axis=0)
    attn = np.concatenate([results[i]["attn"] for i in range(N_CORES)], axis=0)
    covn = np.concatenate(
        [results[i]["coverage_next"] for i in range(N_CORES)], axis=0
    )
    return c_t, attn, covn


def kernel(**inputs):
    nc = _get_nc()
    in_maps = make_in_maps(inputs)
    res = run_bass_kernel_spmd(nc, in_maps, core_ids=list(range(N_CORES)))
    return gather_outputs(res.results)


# revision 5
# speedup vs baseline: 1.2853x; 1.0948x over previous
"""Trainium2 Bass kernel v3 for pointer-generator additive attention.

v2 (transposed pass-1, PE score-reduce) reached HW correctness but ran at
~188us: the PE never held its 2.4 GHz p-state because att(k,c) WAR-stalled
on tanh(k,c-2) every slot (2 PSUM att buffers) and score(k,c-2) waits were
satisfied just-in-time.  A back-to-back microbench shows the PE sustains
218 ns per 512-col matmul (full clock) and fp8 DoubleRow streams TWO
k-tiles in that same 218 ns, with LDWEIGHTS fully hidden.

v3 keeps v2's math but rebuilds the pipeline for PE continuity:
  - 3 att PSUM buffers [128,1024] (6 banks) -- att(k,c) now WARs on
    tanh(k,c-3), finished ~2 slots earlier.  The PSUM transpose scratch
    that blocked the 3rd buffer is gone:
      * attn one-hot columns come from a DMA xbar transpose
        ([16,128] bf16 -> [128,16]; aT[p,c] = attn[c*128+p]), fed by a
        bf16 copy of attn produced inside softmax.
      * dec transposes go through a borrowed att_ps[2] column region
        at startup, drained by ONE [128,64] DVE copy into decT.
  - score lag deepened to 4 slots; th ring = 6 bufs.
  - c_t lag 3 for batches 0-5; {6,7} get singleton softmaxes (batch 6 at
    packed row 96, batch 7 in standalone tiles) so ct(6) runs step 8 and
    ct(7) step 9 -- tail ~12us instead of ~34us.
  - ACT table preloaded via a dummy tanh at t=0.
  - shared PSUM [98,1024]: c_t rows 0-7, score rows 32/64 by parity.
"""

import sys

if "/opt/trn_rl_repo" not in sys.path:
    sys.path.insert(0, "/opt/trn_rl_repo")

import ml_dtypes
import numpy as np

import concourse.bass as bass
import concourse.mybir as mybir
import concourse.tile as tile
from concourse import bacc
from concourse.bass_utils import run_bass_kernel_spmd
from concourse.masks import make_identity

F32 = mybir.dt.float32
BF16 = mybir.dt.bfloat16
FP8 = mybir.dt.float8e4
AF = mybir.ActivationFunctionType
ALU = mybir.AluOpType
DR = mybir.MatmulPerfMode.DoubleRow

CUR = [""]   # emission label, for schedule debugging

N_CORES = 8
B = 64
NB = B // N_CORES  # 8 local batches per core
T = 1024
N = 1024
NCH = 8            # n-chunks of 128 per batch


def build_bass(nb: int = NB) -> bass.Bass:
    nc = bacc.Bacc()

    ef_d = nc.declare_dram_parameter("ef_blk", [nb, 9, 128, T], FP8, isOutput=False)
    lhs_d = nc.declare_dram_parameter("lhs_blk", [128, 9, 128], FP8, isOutput=False)
    eo_d = nc.declare_dram_parameter("eo_blk", [nb, 8, 128, N], BF16, isOutput=False)
    mk_d = nc.declare_dram_parameter("enc_padding_mask", [nb, T], F32, isOutput=False)
    cv_d = nc.declare_dram_parameter("coverage", [nb, T], F32, isOutput=False)
    wdt_d = nc.declare_dram_parameter("W_d_T", [4, 128, 2048], BF16, isOutput=False)
    st_d = nc.declare_dram_parameter("s_t_hat_T", [N, nb], BF16, isOutput=False)
    bd_d = nc.declare_dram_parameter("b_d", [N], BF16, isOutput=False)
    vt_d = nc.declare_dram_parameter("v_T", [128, 8], BF16, isOutput=False)
    id_d = nc.declare_dram_parameter("ident8", [8, 8], F32, isOutput=False)
    ct_o = nc.declare_dram_parameter("c_t", [nb, N], F32, isOutput=True)
    at_o = nc.declare_dram_parameter("attn", [nb, T], F32, isOutput=True)
    cn_o = nc.declare_dram_parameter("coverage_next", [nb, T], F32, isOutput=True)

    with tile.TileContext(nc) as tc:
        with (
            tc.tile_pool(name="consts", bufs=1) as consts,
            tc.tile_pool(name="efp", bufs=1) as efp,
            tc.tile_pool(name="eop", bufs=1) as eop,
            tc.tile_pool(name="thp", bufs=1) as thp,
            tc.tile_pool(name="psA", bufs=1, space="PSUM") as psA,
            tc.tile_pool(name="psS", bufs=1, space="PSUM") as psS,
        ):
            # ---------------- SBUF tiles ----------------
            ef_bufs = [efp.tile([128, 9, T], FP8, name=f"ef{i}") for i in range(3)]
            eo_bufs = [eop.tile([128, 8, N], BF16, name=f"eo{i}") for i in range(5)]
            th_bufs = [thp.tile([128, N], BF16, name=f"th{i}") for i in range(6)]

            lhs_t = consts.tile([128, 9, 128], FP8)
            sT_all = consts.tile([128, NCH, NB], BF16)
            wpairs = [consts.tile([128, 2, N], BF16, name=f"wp{i}") for i in range(4)]
            bd_b = consts.tile([1, N], BF16)
            vt_sb = consts.tile([128, 8], BF16)
            ones8 = consts.tile([1, NB], BF16)
            ident8 = consts.tile([8, 8], F32)
            dec_rows = consts.tile([NB, N], F32)
            decT = consts.tile([128, 8, NB], BF16)
            dummy = consts.tile([1, 1], BF16)
            # singleton softmax: batch b lives at row 32*(b%4) of set b//4
            # (32-aligned partition bases for engine ops)
            mask_s = [consts.tile([98, T], F32, name=f"mk{i}") for i in range(2)]
            covf_s = [consts.tile([98, T], F32, name=f"cf{i}") for i in range(2)]
            attn_s = [consts.tile([98, T], F32, name=f"at{i}") for i in range(2)]
            attnb_s = [consts.tile([98, T], BF16, name=f"ab{i}") for i in range(2)]
            covn_s = [consts.tile([98, T], F32, name=f"cn{i}") for i in range(2)]
            ssum_s = [consts.tile([98, 1], F32, name=f"ss{i}") for i in range(2)]
            rs_s = [consts.tile([98, 1], F32, name=f"rs{i}") for i in range(2)]
            a16 = [consts.tile([16, 128], BF16, name=f"a16_{i}") for i in range(2)]
            aT16 = [consts.tile([128, 16], BF16, name=f"aT16_{i}") for i in range(2)]
            acw = [consts.tile([128, 8, NB], BF16, name=f"acw{b}") for b in range(NB)]
            ct_sb = consts.tile([NB, N], F32)

            # ---------------- PSUM tiles: 6 + 2 = 8 banks ----------------
            att_ps = [psA.tile([128, N], F32, name=f"att{i}") for i in range(3)]
            shared = psS.tile([98, N], F32)   # ct rows 0-7, score @32 / @64

            # ---------------- startup DMAs ----------------
            nc.sync.dma_start(out=sT_all, in_=st_d.rearrange("(c p) b -> p c b", p=128))
            for c2 in range(4):
                for kk in range(2):
                    q = nc.sync if kk == 0 else nc.gpsimd
                    q.dma_start(
                        out=wpairs[c2][:, kk, :],
                        in_=wdt_d[c2, :, :].rearrange(
                            "p (k n) -> p k n", k=2)[:, kk, :],
                    )
            nc.sync.dma_start(out=bd_b, in_=bd_d[None, :])
            nc.sync.dma_start(out=vt_sb, in_=vt_d[:, :])
            nc.gpsimd.dma_start(out=lhs_t, in_=lhs_d[:, :, :])

            nc.sync.dma_start(out=ident8, in_=id_d[:, :])
            for b in range(4):
                nc.gpsimd.dma_start(
                    out=mask_s[0][32 * b:32 * b + 1, :], in_=mk_d[b:b + 1, :]
                )
                nc.gpsimd.dma_start(
                    out=covf_s[0][32 * b:32 * b + 1, :], in_=cv_d[b:b + 1, :]
                )
            nc.vector.memset(ones8, 1.0)
            # ACT table preload off the critical path
            nc.scalar.activation(dummy, ones8[0:1, 0:1], AF.Tanh)
            for b in range(NB):
                nc.vector.memset(acw[b], 0.0)
            for i in range(2):
                nc.vector.memset(a16[i], 0.0)

            # ---------------- DMA helpers ----------------
            def load_ef(b):
                CUR[0] = f'EF{b}'
                buf = ef_bufs[b % 3]
                nc.sync.dma_start(
                    out=buf,
                    in_=ef_d[b, :, :, :].rearrange("c p t -> p c t"),
                )

            def load_eo(b):
                CUR[0] = f'EO{b}'
                buf = eo_bufs[b % 5]
                nc.sync.dma_start(
                    out=buf, in_=eo_d[b, :, :, :].rearrange("c p n -> p c n")
                )

            # ---------------- compute emitters ----------------
            def emit_att(b, c):
                CUR[0] = f'att{b}_{c}'
                buf = ef_bufs[b % 3]
                ps = att_ps[(8 * b + c) % 3]
                lhs_ap = lhs_t[:, 0:c + 2:(c + 1), :]
                for h in range(2):
                    nc.tensor.matmul(
                        ps[:, h * 512:(h + 1) * 512],
                        lhsT=lhs_ap,
                        rhs=buf[:, c:9:(8 - c), h * 512:(h + 1) * 512],
                        perf_mode=DR, start=True, stop=True,
                        skip_group_check=True,
                    )

            def emit_tanh(b, c):
                CUR[0] = f'tanh{b}_{c}'
                nc.scalar.activation(
                    th_bufs[(8 * b + c) % 6], att_ps[(8 * b + c) % 3], AF.Tanh,
                    bias=decT[:, c, b:b + 1],
                )

            def score_reg(b):
                return shared[32:33, :] if b % 2 == 0 else shared[64:65, :]

            def emit_score(b, c):
                CUR[0] = f'score{b}_{c}'
                th = th_bufs[(8 * b + c) % 6]
                reg = score_reg(b)
                for h in range(2):
                    nc.tensor.matmul(
                        reg[:, h * 512:(h + 1) * 512],
                        lhsT=vt_sb[:, c:c + 1],
                        rhs=th[:, h * 512:(h + 1) * 512],
                        start=(c == 0), stop=(c == 7),
                        skip_group_check=True,
                    )

            def emit_softmax(b):
                CUR[0] = f'softmax{b}'
                i, g = b // 4, slice(32 * (b % 4), 32 * (b % 4) + 1)
                ag, ab = attn_s[i][g, :], attnb_s[i][g, :]
                # exp straight from the score PSUM region: no drain, no gather
                nc.scalar.activation(ag, score_reg(b), AF.Exp)
                nc.vector.scalar_tensor_tensor(
                    out=ag, in0=ag, scalar=1.0, in1=mask_s[i][g, :],
                    op0=ALU.mult, op1=ALU.mult, accum_out=ssum_s[i][g, :],
                )
                nc.vector.reciprocal(rs_s[i][g, :], ssum_s[i][g, :])
                nc.vector.tensor_scalar_mul(ag, ag, rs_s[i][g, :])
                nc.vector.tensor_copy(ab, ag)
                nc.vector.tensor_add(
                    covn_s[i][g, :], covf_s[i][g, :], ag
                )
                nc.gpsimd.dma_start(out=at_o[b:b + 1, :], in_=ag)
                nc.gpsimd.dma_start(out=cn_o[b:b + 1, :], in_=covn_s[i][g, :])

            def emit_acw(b):
                CUR[0] = f'acw{b}'
                i, r = b // 4, 32 * (b % 4)
                t16 = a16[b % 2]
                # regroup + xbar both on sync, issued ahead of the step's bulk
                nc.sync.dma_start(
                    out=t16[0:8, :],
                    in_=attnb_s[i][r:r + 1, :].rearrange("p (c t) -> p c t", c=8),
                )
                aT = aT16[b % 2]
                nc.sync.dma_start_transpose(out=aT, in_=t16)
                nc.vector.tensor_copy(acw[b][:, :, b], aT[:, 0:8])

            def emit_ct(b, c):
                CUR[0] = f'ct{b}_{c}'
                buf = eo_bufs[b % 5]
                for h in range(2):
                    nc.tensor.matmul(
                        shared[0:8, h * 512:(h + 1) * 512],
                        lhsT=acw[b][:, c, :],
                        rhs=buf[:, c, h * 512:(h + 1) * 512],
                        start=(b == 0 and c == 0),
                        stop=(b == NB - 1 and c == 7),
                        skip_group_check=True,
                    )

            def emit_dec():
                CUR[0] = 'dec'
                dv = att_ps[1]   # matvec accumulator (rows 0-7)
                sc = att_ps[2]   # transpose scratch (cols 0-63)
                dT = decT.rearrange("p c b -> p (c b)")
                # h-split: n-halves pipelined so tanh(0,0) starts ~4us sooner
                for h in range(2):
                    for kj in range(NCH):
                        nc.tensor.matmul(
                            dv[0:8, h * 512:(h + 1) * 512],
                            lhsT=sT_all[:, kj, :],
                            rhs=wpairs[kj // 2][:, kj % 2, h * 512:(h + 1) * 512],
                            start=(kj == 0), stop=False, skip_group_check=True,
                        )
                    nc.tensor.matmul(
                        dv[0:8, h * 512:(h + 1) * 512],
                        lhsT=ones8, rhs=bd_b[0:1, h * 512:(h + 1) * 512],
                        start=False, stop=True, skip_group_check=True,
                    )
                    nc.vector.tensor_copy(
                        dec_rows[:, h * 512:(h + 1) * 512],
                        dv[0:8, h * 512:(h + 1) * 512],
                    )
                    for c in range(4 * h, 4 * h + 4):
                        nc.tensor.matmul(
                            sc[:, c * 8:(c + 1) * 8],
                            lhsT=dec_rows[:, c * 128:(c + 1) * 128],
                            rhs=ident8[0:8, 0:8],
                            is_transpose=True, start=True, stop=True,
                            skip_group_check=True,
                        )
                    nc.vector.tensor_copy(
                        dT[:, 32 * h:32 * h + 32],
                        sc[:, 32 * h:32 * h + 32],
                    )

            # ---------------- schedule ----------------
            load_ef(0)
            load_ef(1)
            emit_att(0, 0)
            emit_dec()

            for k in range(10):
                for c in range(8):
                    if k <= 7 and not (k == 0 and c == 0):
                        emit_att(k, c)
                    if k <= 7:
                        emit_tanh(k, c)
                    # score lag 4: slots 0-3 finish batch k-1, 4-7 run batch
                    # k; batch 7 runs at lag 2 so the tail starts sooner
                    if c < 4:
                        sb_, sc_ = k - 1, 4 + c
                    else:
                        sb_, sc_ = k, c - 4
                    if 0 <= sb_ <= 6 and (sb_ == k - 1 or k <= 7):
                        emit_score(sb_, sc_)
                    if k == 7 and c >= 2:
                        emit_score(7, c - 2)
                    if c == 7 and 1 <= k <= 7:
                        emit_softmax(k - 1)
                        emit_acw(k - 1)
                    # c_t: lag 4 (acw chain gets ~1.5 steps of slack)
                    if 4 <= k <= 7:
                        emit_ct(k - 4, c)
                # tail: batch-7 softmax chain first, then remaining c_t
                if k == 8:
                    emit_score(7, 6)
                    emit_score(7, 7)
                    emit_softmax(7)
                    emit_acw(7)
                    for b_ in (4, 5, 6):
                        for c_ in range(8):
                            emit_ct(b_, c_)
                if k == 9:
                    for c_ in range(8):
                        emit_ct(7, c_)
                # bulk loads at end of step: latency-critical smalls (score
                # gather, acw xbar) issued mid-step go ahead of them in the
                # sync queue; prefetch depth still covers arrival
                if k + 2 <= NB - 1:
                    load_ef(k + 2)
                # EO shifted one step early: WAR on buf (k+1)%4 vs ct(k-3)
                # is emitted just above, and the tail then has no bulk DMA
                # blocking the acw smalls
                if k == 0:
                    load_eo(0)
                    load_eo(1)
                elif k + 1 <= NB - 1:
                    load_eo(k + 1)
                if k == 0:
                    for b in range(4, 8):
                        nc.gpsimd.dma_start(
                            out=mask_s[1][32 * (b % 4):32 * (b % 4) + 1, :],
                            in_=mk_d[b:b + 1, :],
                        )
                        nc.gpsimd.dma_start(
                            out=covf_s[1][32 * (b % 4):32 * (b % 4) + 1, :],
                            in_=cv_d[b:b + 1, :],
                        )

            nc.vector.tensor_copy(ct_sb, shared[0:8, :])
            nc.sync.dma_start(out=ct_o[:, :], in_=ct_sb)

    nc.finalize()
    return nc


_CACHE: dict = {}


def _get_nc() -> bass.Bass:
    if "nc" not in _CACHE:
        _CACHE["nc"] = build_bass(NB)
    return _CACHE["nc"]


def make_in_maps(inputs: dict) -> list:
    f = lambda x: np.ascontiguousarray(np.asarray(x), dtype=np.float32)
    s = f(inputs["s_t_hat"])
    eo = f(inputs["encoder_outputs"])
    ef = f(inputs["encoder_feature"]).reshape(B, T, N)
    mk = f(inputs["enc_padding_mask"])
    cv = f(inputs["coverage"])
    wdt = np.ascontiguousarray(f(inputs["W_d"]).T).astype(ml_dtypes.bfloat16)
    wdt_pairs = np.ascontiguousarray(
        wdt.reshape(8, 128, N).transpose(1, 0, 2).reshape(128, 4, 2048)
        .transpose(1, 0, 2)
    )
    bd = f(inputs["b_d"]).astype(ml_dtypes.bfloat16)
    wc = f(inputs["W_c"])
    vv = f(inputs["v"])
    vt = np.ascontiguousarray(vv.reshape(8, 128).T).astype(ml_dtypes.bfloat16)

    lhs = np.zeros((128, 9, 128), np.float32)
    lhs[:, 0, :] = np.eye(128, dtype=np.float32)
    for c in range(8):
        lhs[0, c + 1, :] = wc[c * 128:(c + 1) * 128]
    lhs_blk = lhs.astype(ml_dtypes.float8_e4m3)

    in_maps = []
    for i in range(N_CORES):
        sl = slice(i * NB, (i + 1) * NB)
        ef_blk = np.zeros((NB, 9, 128, T), ml_dtypes.float8_e4m3)
        ef_blk[:, 0:8] = ef[sl].transpose(0, 2, 1).reshape(
            NB, NCH, 128, T).astype(ml_dtypes.float8_e4m3)
        ef_blk[:, 8, 0, :] = cv[sl].astype(ml_dtypes.float8_e4m3)
        eo_blk = np.ascontiguousarray(
            eo[sl].reshape(NB, 8, 128, N)
        ).astype(ml_dtypes.bfloat16)
        in_maps.append({
            "ef_blk": ef_blk,
            "lhs_blk": lhs_blk,
            "eo_blk": eo_blk,
            "enc_padding_mask": mk[sl],
            "coverage": cv[sl],
            "W_d_T": wdt_pairs,
            "s_t_hat_T": np.ascontiguousarray(s[sl].T).astype(ml_dtypes.bfloat16),
            "b_d": bd,
            "v_T": vt,
            "ident8": np.eye(8, dtype=np.float32),
        })
    return in_maps


def gather_outputs(results: list):
    c_t = np.concatenate([results[i]["c_t"] for i in range(N_CORES)], axis=0)
    attn = np.concatenate([results[i]["attn"] for i in range(N_CORES)], axis=0)
    covn = np.concatenate(
        [results[i]["coverage_next"] for i in range(N_CORES)], axis=0
    )
    return c_t, attn, covn


def kernel(**inputs):
    nc = _get_nc()
    in_maps = make_in_maps(inputs)
    res = run_bass_kernel_spmd(nc, in_maps, core_ids=list(range(N_CORES)))
    return gather_outputs(res.results)
